# revision 13
# baseline (speedup 1.0000x reference)
"""Trainium2 Bass kernel for nn_GatedQuestionAnswering.

Model: bidirectional GRU encoder (fwd full 512 steps; bwd collapses to ONE
cell step because reference uses bwd_hs[-1] = cell(x[511], h0)), then a
1024-step decoder GRU (hidden 1024), then vocab projection [1024,1024] @
[1024, 28996].

Strategy (8 cores, SPMD, no collectives):
  - serial recurrences run redundantly on every core (latency-bound);
    cross-core collectives have a ~10us floor so per-step exchange loses
  - vocab projection is column-sharded: W_pred padded to [1024, 32768],
    each core computes a [1024, 4096] slice
  - all weight transposes / bias folding / padding done host-side in numpy
  - recurrence matvecs: W_hh^T stationary in fp8(e3m4, x128 scale) for
    faster LDWEIGHTS (the matvec bottleneck); h streamed as bf16 (/128)
    [128,1] columns; gates land in PSUM as [128, m] so the GRU gate
    elementwise math is lane-parallel
  - GX gemms + vocab projection in bf16 (fp32 matmul is 4 cyc/row vs 1)
  - decoder h history stored k-major [128, k*(T+1)+t] in scaled bf16 so the
    projection's stationary tiles are CONTIGUOUS (FWL reads weights
    contiguously; a strided bf16 stationary operand faults the PE)
  - W_pred pre-scaled x128 so the h scale cancels
"""

import os
import sys

for _p in ("/opt/trn_rl_repo",):
    if _p not in sys.path:
        sys.path.insert(0, _p)

import numpy as np
import ml_dtypes

import concourse.bass as bass
import concourse.mybir as mybir
import concourse.tile as tile
from concourse import bacc
from concourse.bass import ds

AF = mybir.ActivationFunctionType
ALU = mybir.AluOpType
F32 = mybir.dt.float32
BF16 = mybir.dt.bfloat16
FP8 = mybir.dt.float8e3

I = 768
KP = 896          # 768 + 1 bias row, zero-padded to 7*128
H = 512           # encoder hidden
G = 3 * H         # 1536
BI = 1024         # decoder hidden
G2 = 3 * BI       # 3072
V = 28996
VP = 32768        # padded vocab (8 * 4096)
NCORES = 8
VC = VP // NCORES  # 4096

REC_FP8 = True              # fp8 e3m4 recurrence weights (else bf16)
HSCALE = 128.0 if REC_FP8 else 1.0
RW_DT = FP8 if REC_FP8 else BF16
RW_NP = ml_dtypes.float8_e3m4 if REC_FP8 else ml_dtypes.bfloat16
# keep-hot filler matmuls per step: cover the elementwise tail so the PE
# never idles (idle gaps drop the HAM clock gate to half rate)
DUMMY_ENC = int(os.environ.get("BK_DUMMY_ENC", "24"))
DUMMY_DEC = int(os.environ.get("BK_DUMMY_DEC", "36"))


def _gemm_gx(nc, pool_psum, wt_sb, xt_sb, gx_sb, n_steps, gdiv):
    """gx in t-major [128, t*gdiv + m] bf16 layout from bf16 wt/xt tiles.
    wt_sb: [128, 7, n_gates], xt_sb: [128, 7, n_steps], gx_sb: [128, n_steps*gdiv].
    """
    gxv = gx_sb[:].rearrange("p (t m) -> p t m", m=gdiv)
    n_blocks = (n_steps + 511) // 512
    for m in range(gdiv):
        for nb in range(n_blocks):
            t0 = nb * 512
            tn = min(512, n_steps - t0)
            ps = pool_psum.tile([128, 512], F32, tag="gxps")
            for k in range(7):
                nc.tensor.matmul(
                    ps[:, :tn],
                    wt_sb[:, k, m * 128:(m + 1) * 128],
                    xt_sb[:, k, t0:t0 + tn],
                    start=(k == 0),
                    stop=(k == 6),
                )
            # strided store into t-major bf16 GX
            nc.scalar.activation(gxv[:, t0:t0 + tn, m:m + 1], ps[:, :tn], AF.Copy)


def build_program(bhn_e_np, bhn_d_np, lsteps=512, losteps=1024,
                  enc_unroll=4, dec_unroll=4):
    nc = bacc.Bacc("TRN2", target_bir_lowering=False, debug=False,
                   num_devices=NCORES)

    xte = nc.dram_tensor("xte", [KP, lsteps], BF16, kind="ExternalInput")
    wte = nc.dram_tensor("wte", [KP, G], BF16, kind="ExternalInput")
    wtb = nc.dram_tensor("wtb", [KP, G], BF16, kind="ExternalInput")
    xtd = nc.dram_tensor("xtd", [KP, losteps], BF16, kind="ExternalInput")
    wtd = nc.dram_tensor("wtd", [KP, G2], BF16, kind="ExternalInput")
    whe = nc.dram_tensor("whe", [H, G], RW_DT, kind="ExternalInput")
    whd = nc.dram_tensor("whd", [BI, G2], RW_DT, kind="ExternalInput")
    wp = nc.dram_tensor("wp", [BI, VC], BF16, kind="ExternalInput")
    out = nc.dram_tensor("out", [losteps, VC], F32, kind="ExternalOutput")

    use_bhn_e = bhn_e_np is not None and np.any(bhn_e_np)
    use_bhn_d = bhn_d_np is not None and np.any(bhn_d_np)
    bhe_d = nc.inline_tensor(
        np.ascontiguousarray(bhn_e_np.reshape(4, 128).T), name="bhe") if use_bhn_e else None
    bhd_d = nc.inline_tensor(
        np.ascontiguousarray(bhn_d_np.reshape(8, 128).T), name="bhd") if use_bhn_d else None

    ENC_T = lsteps
    DEC_T = losteps
    NS = DEC_T // 128  # projection step-tiles
    INV_S = 1.0 / HSCALE
    TP = DEC_T + 1     # h history slots per k-block

    with tile.TileContext(nc) as tc:
        with tc.tile_pool(name="persist", bufs=1) as pp, \
             tc.tile_pool(name="psum_dum", bufs=1, space="PSUM") as pdum:
            # persistent state
            # hp[p, k*TP + t] = h_{t-1}[k*128 + p] / HSCALE  (bf16); slot 0 = h0
            hp = pp.tile([128, 8 * TP], BF16, tag="hp")
            ps_dum = pdum.tile([128, 4], F32, tag="ps_dum")
            h32 = [pp.tile([128, 8], F32, tag=f"h32_{j}", name=f"h32_{j}")
                   for j in range(2)]
            he_f = [pp.tile([128, 4], F32, tag=f"he_f{j}", name=f"he_f{j}")
                    for j in range(2)]
            he_b = [pp.tile([128, 4], BF16, tag=f"he_b{j}", name=f"he_b{j}")
                    for j in range(2)]
            whd_sb = pp.tile([128, 8, G2], RW_DT, tag="whd_sb")
            gxd = pp.tile([128, DEC_T * 24], BF16, tag="gxd")
            bhe_sb = pp.tile([128, 4], F32, tag="bhe_sb") if use_bhn_e else None
            bhd_sb = pp.tile([128, 8], F32, tag="bhd_sb") if use_bhn_d else None
            nc.sync.dma_start(
                whd_sb[:], whd.ap().rearrange("(ko ki) g -> ki ko g", ki=128))
            if use_bhn_e:
                nc.sync.dma_start(bhe_sb[:], bhe_d[:, :])
            if use_bhn_d:
                nc.sync.dma_start(bhd_sb[:], bhd_d[:, :])

            # ---------------- Phase A: encoder GX + bw single cell ----------
            with tc.tile_pool(name="encgx", bufs=1) as pa, \
                 tc.tile_pool(name="psum_enc", bufs=2, space="PSUM") as prec:
                gxe = pa.tile([128, ENC_T * 12], BF16, tag="gxe")
                whe_sb = pa.tile([128, 4, G], RW_DT, tag="whe_sb")
                xte_sb = pa.tile([128, 7, ENC_T], BF16, tag="xte_sb")
                with tc.tile_pool(name="encw", bufs=1) as pw, \
                     tc.tile_pool(name="psum_ga", bufs=1, space="PSUM") as pgx:
                    wte_sb = pw.tile([128, 7, G], BF16, tag="wte_sb")
                    wtb_sb = pw.tile([128, 7, G], BF16, tag="wtb_sb")
                    nc.sync.dma_start(
                        xte_sb[:], xte.ap().rearrange("(ko ki) t -> ki ko t", ki=128))
                    nc.sync.dma_start(
                        wte_sb[:], wte.ap().rearrange("(ko ki) g -> ki ko g", ki=128))
                    nc.sync.dma_start(
                        wtb_sb[:], wtb.ap().rearrange("(ko ki) g -> ki ko g", ki=128))
                    nc.sync.dma_start(
                        whe_sb[:], whe.ap().rearrange("(ko ki) g -> ki ko g", ki=128))

                    _gemm_gx(nc, pgx, wte_sb, xte_sb, gxe, ENC_T, 12)

                    # backward encoder: single cell on x[last], h0 = 0
                    # gx_b = wtb_aug.T @ x_aug[:, last]  -> [128, 12] psum
                    ps_b = pgx.tile([128, 12], F32, tag="ps_b")
                    for m in range(12):
                        for k in range(7):
                            nc.tensor.matmul(
                                ps_b[:, m:m + 1],
                                wtb_sb[:, k, m * 128:(m + 1) * 128],
                                xte_sb[:, k, ENC_T - 1:ENC_T],
                                start=(k == 0),
                                stop=(k == 6),
                            )
                    # z = sigmoid(gx_z); n = tanh(gx_n [+ r*bhn]); h = (1-z)*n
                    zb = pa.tile([128, 4], F32, tag="zb")
                    nb = pa.tile([128, 4], F32, tag="nb")
                    tb = pa.tile([128, 4], F32, tag="tb")
                    nc.scalar.activation(zb[:], ps_b[:, 4:8], AF.Sigmoid)
                    if use_bhn_e:
                        rb = pa.tile([128, 4], F32, tag="rb")
                        nc.scalar.activation(rb[:], ps_b[:, 0:4], AF.Sigmoid)
                        nc.vector.tensor_mul(tb[:], rb[:], bhe_sb[:])
                        nc.vector.tensor_add(tb[:], tb[:], ps_b[:, 8:12])
                        nc.scalar.activation(nb[:], tb[:], AF.Tanh)
                    else:
                        nc.scalar.activation(nb[:], ps_b[:, 8:12], AF.Tanh)
                    nc.vector.tensor_scalar(
                        out=zb[:], in0=zb[:], scalar1=-1.0, scalar2=1.0,
                        op0=ALU.mult, op1=ALU.add)
                    # h_bw -> h32[0] cols 4:8
                    nc.vector.tensor_mul(h32[0][:, 4:8], zb[:], nb[:])

                # ---------------- Phase B: forward encoder recurrence -------
                nc.vector.memset(he_f[0][:], 0.0)
                nc.vector.memset(he_b[0][:], 0.0)
                gxev = gxe[:]
                with tc.For_i(0, ENC_T, enc_unroll,
                              hint_engines=(mybir.EngineType.PE,)) as iv:
                    for u in range(enc_unroll):
                        t = iv + u
                        cur = u % 2
                        nxt = (u + 1) % 2
                        ps_rz = prec.tile([128, 8], F32, tag="ps_rz_e")
                        ps_n = prec.tile([128, 4], F32, tag="ps_n_e")
                        for m in range(12):
                            tgt = ps_rz[:, m:m + 1] if m < 8 else ps_n[:, m - 8:m - 7]
                            for k in range(4):
                                nc.tensor.matmul(
                                    tgt,
                                    whe_sb[:, k, m * 128:(m + 1) * 128],
                                    he_b[cur][:, k:k + 1],
                                    start=(k == 0), stop=(k == 3))
                        grz = pa.tile([128, 8], F32, tag=f"grz_e{u}")
                        rz = pa.tile([128, 8], F32, tag=f"rz_e{u}")
                        aa = pa.tile([128, 4], F32, tag=f"aa_e{u}")
                        omz = pa.tile([128, 4], F32, tag=f"omz_e{u}")
                        t1 = pa.tile([128, 4], F32, tag=f"t1_e{u}")
                        bn = pa.tile([128, 4], F32, tag=f"bn_e{u}")
                        nn = pa.tile([128, 4], F32, tag=f"nn_e{u}")
                        for j in range(DUMMY_ENC):
                            nc.tensor.matmul(
                                ps_dum[:, j % 4:j % 4 + 1],
                                whe_sb[:, 0, 0:128], gxev[:, 0:1],
                                start=True, stop=True)
                        nc.vector.tensor_add(grz[:], ps_rz[:], gxev[:, ds(t * 12, 8)])
                        nc.scalar.activation(rz[:], grz[:], AF.Sigmoid)
                        # early (overlaps n-gate matvecs): a = z*h, 1-z
                        nc.vector.tensor_mul(aa[:], rz[:, 4:8], he_f[cur][:])
                        nc.vector.tensor_scalar(
                            out=omz[:], in0=rz[:, 4:8], scalar1=-1.0, scalar2=1.0,
                            op0=ALU.mult, op1=ALU.add)
                        if use_bhn_e:
                            nc.vector.tensor_add(ps_n[:], ps_n[:], bhe_sb[:])
                        nc.vector.tensor_mul(t1[:], rz[:, 0:4], ps_n[:])
                        nc.vector.tensor_add(t1[:], t1[:], gxev[:, ds(t * 12 + 8, 4)])
                        nc.scalar.activation(nn[:], t1[:], AF.Tanh)
                        # h_new = (1-z)*n + z*h ; scaled bf16 copy unblocks PE
                        nc.vector.tensor_mul(bn[:], omz[:], nn[:])
                        nc.vector.tensor_add(he_f[nxt][:], bn[:], aa[:])
                        nc.vector.tensor_scalar_mul(he_b[nxt][:], he_f[nxt][:], INV_S)
                # final fw state is in he_f[0] (ENC_T multiple of 2)
                nc.vector.tensor_copy(h32[0][:, 0:4], he_f[0][:])

            # h0 for decoder: scaled bf16 into hp slot 0 (stride TP)
            nc.vector.tensor_scalar_mul(hp[:, ds(0, 8, TP)], h32[0][:], INV_S)

            # ---------------- Phase C: decoder GX ---------------------------
            with tc.tile_pool(name="decgx", bufs=1) as pc, \
                 tc.tile_pool(name="wtd_stream", bufs=3) as pwtd, \
                 tc.tile_pool(name="psum_gc", bufs=2, space="PSUM") as pgx:
                xtd_sb = pc.tile([128, 7, DEC_T], BF16, tag="xtd_sb")
                nc.sync.dma_start(
                    xtd_sb[:], xtd.ap().rearrange("(ko ki) t -> ki ko t", ki=128))
                gxdv = gxd[:].rearrange("p (t m) -> p t m", m=24)
                n_blocks = (DEC_T + 511) // 512
                for m in range(24):
                    wtd_t = pwtd.tile([128, 7, 128], BF16, tag="wtd_t")
                    nc.sync.dma_start(
                        wtd_t[:],
                        wtd.ap()[:, m * 128:(m + 1) * 128].rearrange(
                            "(ko ki) g -> ki ko g", ki=128))
                    for nb in range(n_blocks):
                        t0 = nb * 512
                        tn = min(512, DEC_T - t0)
                        ps = pgx.tile([128, 512], F32, tag="gxps")
                        for k in range(7):
                            nc.tensor.matmul(
                                ps[:, :tn], wtd_t[:, k, :], xtd_sb[:, k, t0:t0 + tn],
                                start=(k == 0), stop=(k == 6))
                        nc.scalar.activation(gxdv[:, t0:t0 + tn, m:m + 1],
                                             ps[:, :tn], AF.Copy)

            # ---------------- Phase D: decoder recurrence -------------------
            with tc.tile_pool(name="dec_sc", bufs=1) as pd, \
                 tc.tile_pool(name="psum_dec", bufs=2, space="PSUM") as prec:
                with tc.For_i(0, DEC_T, dec_unroll,
                              hint_engines=(mybir.EngineType.PE,)) as iv:
                    for u in range(dec_unroll):
                        t = iv + u
                        cur = u % 2
                        nxt = (u + 1) % 2
                        ps_rz = prec.tile([128, 16], F32, tag="ps_rz_d")
                        ps_n = prec.tile([128, 8], F32, tag="ps_n_d")
                        for m in range(24):
                            tgt = (ps_rz[:, m:m + 1] if m < 16
                                   else ps_n[:, m - 16:m - 15])
                            for k in range(8):
                                nc.tensor.matmul(
                                    tgt,
                                    whd_sb[:, k, m * 128:(m + 1) * 128],
                                    hp[:, ds(k * TP + t, 1)],
                                    start=(k == 0), stop=(k == 7))
                        grz = pd.tile([128, 16], F32, tag=f"grz_d{u}")
                        rz = pd.tile([128, 16], F32, tag=f"rz_d{u}")
                        aa = pd.tile([128, 8], F32, tag=f"aa_d{u}")
                        a1 = pd.tile([128, 8], F32, tag=f"a1_d{u}")
                        omz = pd.tile([128, 8], F32, tag=f"omz_d{u}")
                        t1 = pd.tile([128, 8], F32, tag=f"t1_d{u}")
                        bn = pd.tile([128, 8], F32, tag=f"bn_d{u}")
                        nn = pd.tile([128, 8], F32, tag=f"nn_d{u}")
                        for j in range(DUMMY_DEC):
                            nc.tensor.matmul(
                                ps_dum[:, j % 4:j % 4 + 1],
                                whd_sb[:, 0, 0:128], gxd[:][:, 0:1],
                                start=True, stop=True)
                        nc.vector.tensor_add(grz[:], ps_rz[:],
                                             gxd[:][:, ds(t * 24, 16)])
                        nc.scalar.activation(rz[:], grz[:], AF.Sigmoid)
                        # early (overlap n matvecs): a = z*h, a1 = a/S, 1-z
                        nc.vector.tensor_mul(aa[:], rz[:, 8:16], h32[cur][:])
                        nc.vector.tensor_scalar_mul(a1[:], aa[:], INV_S)
                        nc.vector.tensor_scalar(
                            out=omz[:], in0=rz[:, 8:16], scalar1=-1.0,
                            scalar2=1.0, op0=ALU.mult, op1=ALU.add)
                        if use_bhn_d:
                            nc.vector.tensor_add(ps_n[:], ps_n[:], bhd_sb[:])
                        nc.vector.tensor_mul(t1[:], rz[:, 0:8], ps_n[:])
                        nc.vector.tensor_add(t1[:], t1[:],
                                             gxd[:][:, ds(t * 24 + 16, 8)])
                        nc.scalar.activation(nn[:], t1[:], AF.Tanh)
                        # critical: hp[t+1] = bn/S + a1 unblocks next step
                        nc.vector.tensor_mul(bn[:], omz[:], nn[:])
                        nc.vector.scalar_tensor_tensor(
                            hp[:, ds(t + 1, 8, TP)], bn[:], INV_S, a1[:],
                            ALU.mult, ALU.add)
                        # off-critical: f32 h carry
                        nc.vector.tensor_add(h32[nxt][:], bn[:], aa[:])

            # ---------------- Phase E: vocab projection ---------------------
            with tc.tile_pool(name="wp_pool", bufs=2) as pwp, \
                 tc.tile_pool(name="out_pool", bufs=3) as pout, \
                 tc.tile_pool(name="psum_o", bufs=4, space="PSUM") as pso:
                for n in range(VC // 512):
                    wpn = pwp.tile([128, 8, 512], BF16, tag="wpn")
                    nc.sync.dma_start(
                        wpn[:],
                        wp.ap()[:, n * 512:(n + 1) * 512].rearrange(
                            "(ko ki) v -> ki ko v", ki=128))
                    for s in range(NS):
                        ps = pso.tile([128, 512], F32, tag="ps_o")
                        for k in range(8):
                            # contiguous bf16 stationary tile (FWL-safe)
                            nc.tensor.matmul(
                                ps[:],
                                hp[:, k * TP + 1 + s * 128:
                                   k * TP + 1 + (s + 1) * 128],
                                wpn[:, k, :],
                                start=(k == 0), stop=(k == 7))
                        ot = pout.tile([128, 512], F32, tag="ot")
                        nc.vector.tensor_copy(ot[:], ps[:])
                        nc.sync.dma_start(
                            out.ap()[s * 128:(s + 1) * 128,
                                     n * 512:(n + 1) * 512], ot[:])

    nc.compile()
    return nc


def _prep_inputs(inputs, lsteps=512, losteps=1024):
    f = lambda k: np.asarray(inputs[k], np.float32)
    bf = lambda a: np.ascontiguousarray(a).astype(ml_dtypes.bfloat16)
    x = f("input_context")[:lsteps]
    oc = f("output_context")[:losteps]
    dec_in = np.concatenate([oc[:1], oc[:-1]], axis=0)

    def aug_x(xT_cols):
        a = np.zeros((KP, xT_cols.shape[1]), np.float32)
        a[:I] = xT_cols
        a[I] = 1.0
        return a

    def aug_w(wih, bih, bhh, hh):
        a = np.zeros((KP, 3 * hh), np.float32)
        a[:I] = wih.T
        bias = bih.copy()
        bias[:2 * hh] += bhh[:2 * hh]
        a[I] = bias
        return a

    def rq(whh):  # recurrence weight: transpose, scale, quantize
        w = np.ascontiguousarray(whh.T) * HSCALE
        if REC_FP8:
            w = np.clip(w, -15.5, 15.5)
        return w.astype(RW_NP)

    xte = bf(aug_x(x.T))
    xtd = bf(aug_x(dec_in.T))
    wte = bf(aug_w(f("fw_wih"), f("fw_bih"), f("fw_bhh"), H))
    wtb = bf(aug_w(f("bw_wih"), f("bw_bih"), f("bw_bhh"), H))
    wtd = bf(aug_w(f("dec_wih"), f("dec_bih"), f("dec_bhh"), BI))
    whe = rq(f("fw_whh"))
    whd = rq(f("dec_whh"))
    wp_pad = np.zeros((BI, VP), np.float32)
    wp_pad[:, :V] = f("W_pred") * HSCALE
    wp_pad = wp_pad.astype(ml_dtypes.bfloat16)
    bhn_e = f("fw_bhh")[2 * H:]
    bhn_d = f("dec_bhh")[2 * BI:]

    common = dict(xte=xte, wte=wte, wtb=wtb, xtd=xtd, wtd=wtd, whe=whe,
                  whd=whd)
    in_maps = [dict(common, wp=np.ascontiguousarray(wp_pad[:, c * VC:(c + 1) * VC]))
               for c in range(NCORES)]
    return in_maps, bhn_e, bhn_d


def _bench_pjrt(nc, in_maps, warmup=2, iters=10):
    """Time repeated on-device executions with device-resident inputs.

    Returns (per_call_min_s, burst_avg_s): per-call = block every call
    (exec + RTT); burst = async dispatch of `iters` calls, block once
    (amortizes dispatch; ~max(exec, dispatch)).
    """
    import time

    import jax
    from jax.sharding import Mesh, PartitionSpec
    from jax.experimental.shard_map import shard_map

    from concourse import bass2jax
    from concourse.bass2jax import _bass_exec_p, install_neuronx_cc_hook

    install_neuronx_cc_hook()
    partition_name = (nc.partition_id_tensor.name
                      if nc.partition_id_tensor else None)
    in_names, out_names, out_avals, zero_outs = [], [], [], []
    for alloc in nc.m.functions[0].allocations:
        if not isinstance(alloc, mybir.MemoryLocationSet):
            continue
        name = alloc.memorylocations[0].name
        if alloc.kind == "ExternalInput":
            if name != partition_name:
                in_names.append(name)
        elif alloc.kind == "ExternalOutput":
            shape = tuple(alloc.tensor_shape)
            dtype = mybir.dt.np(alloc.dtype)
            out_names.append(name)
            out_avals.append(jax.core.ShapedArray(shape, dtype))
            zero_outs.append(np.zeros(shape, dtype))
    n_params = len(in_names)
    all_names = list(in_names) + list(out_names)
    if partition_name is not None:
        all_names.append(partition_name)

    def _body(*args):
        operands = list(args)
        if partition_name is not None:
            operands.append(bass2jax.partition_id_tensor())
        return tuple(_bass_exec_p.bind(
            *operands,
            out_avals=tuple(out_avals),
            in_names=tuple(all_names),
            out_names=tuple(out_names),
            lowering_input_output_aliases=(),
            sim_require_finite=False,
            sim_require_nnan=False,
            nc=nc,
        ))

    devices = jax.devices()[:NCORES]
    mesh = Mesh(np.asarray(devices), ("core",))
    nin = n_params + len(zero_outs)
    fn = jax.jit(shard_map(
        _body, mesh=mesh,
        in_specs=(PartitionSpec("core"),) * nin,
        out_specs=(PartitionSpec("core"),) * len(out_names),
        check_rep=False))
    sh = jax.sharding.NamedSharding(mesh, PartitionSpec("core"))
    dev_in = [
        jax.device_put(
            np.concatenate([np.asarray(in_maps[c][nm]) for c in range(NCORES)],
                           axis=0), sh)
        for nm in in_names
    ] + [
        jax.device_put(
            np.zeros((NCORES * z.shape[0], *z.shape[1:]), z.dtype), sh)
        for z in zero_outs
    ]
    for _ in range(warmup):
        jax.block_until_ready(fn(*dev_in))
    times = []
    for _ in range(iters):
        t0 = time.perf_counter()
        jax.block_until_ready(fn(*dev_in))
        times.append(time.perf_counter() - t0)
    t0 = time.perf_counter()
    outs = None
    for _ in range(iters):
        outs = fn(*dev_in)
    jax.block_until_ready(outs)
    burst = (time.perf_counter() - t0) / iters
    return min(times), burst


_CACHE = {}
LAST_EXEC_NS = None


def kernel(**inputs) -> np.ndarray:
    global LAST_EXEC_NS

    from concourse import bass_utils

    in_maps, bhn_e, bhn_d = _prep_inputs(inputs)
    key = (bool(np.any(bhn_e)), bool(np.any(bhn_d)))
    if key not in _CACHE:
        _CACHE[key] = build_program(bhn_e, bhn_d)
    nc = _CACHE[key]
    res = bass_utils.run_bass_kernel_spmd(
        nc, in_maps, core_ids=list(range(NCORES)))
    LAST_EXEC_NS = res.exec_time_ns
    if os.environ.get("BASS_KERNEL_BENCH", "0") == "1":
        per_call, burst = _bench_pjrt(nc, in_maps)
        LAST_EXEC_NS = int(burst * 1e9)
        print(f"bench: per-call min {per_call*1e3:.3f} ms, "
              f"burst avg {burst*1e3:.3f} ms")
    preds = np.concatenate([res.results[c]["out"] for c in range(NCORES)],
                           axis=1)
    return np.ascontiguousarray(preds[:, :V]).astype(np.float32)


if __name__ == "__main__":
    # smoke test with random inputs
    rng = np.random.default_rng(0)
    inp = {
        "input_context": rng.standard_normal((512, I), dtype=np.float32),
        "output_context": rng.standard_normal((1024, I), dtype=np.float32),
        "fw_wih": rng.standard_normal((G, I), dtype=np.float32) * 0.02,
        "fw_whh": rng.standard_normal((G, H), dtype=np.float32) * 0.02,
        "fw_bih": np.zeros(G, np.float32), "fw_bhh": np.zeros(G, np.float32),
        "bw_wih": rng.standard_normal((G, I), dtype=np.float32) * 0.02,
        "bw_whh": rng.standard_normal((G, H), dtype=np.float32) * 0.02,
        "bw_bih": np.zeros(G, np.float32), "bw_bhh": np.zeros(G, np.float32),
        "dec_wih": rng.standard_normal((G2, I), dtype=np.float32) * 0.02,
        "dec_whh": rng.standard_normal((G2, BI), dtype=np.float32) * 0.02,
        "dec_bih": np.zeros(G2, np.float32), "dec_bhh": np.zeros(G2, np.float32),
        "W_pred": rng.standard_normal((BI, V), dtype=np.float32) * 0.02,
    }
    out = kernel(**inp)
    print("out", out.shape, out.dtype, float(np.abs(out).max()))


# revision 18
# speedup vs baseline: 2.2232x; 2.2232x over previous
"""Trainium2 Bass kernel for nn_GatedQuestionAnswering.

Model: bidirectional GRU encoder (fwd full 512 steps; bwd collapses to ONE
cell step because reference uses bwd_hs[-1] = cell(x[511], h0)), then a
1024-step decoder GRU (hidden 1024), then vocab projection [1024,1024] @
[1024, 28996].

Strategy (8 cores, SPMD, no collectives):
  - serial recurrences run redundantly on every core (latency-bound);
    cross-core collectives have a ~10us floor so per-step exchange loses
  - vocab projection is column-sharded: W_pred padded to [1024, 32768],
    each core computes a [1024, 4096] slice
  - all weight transposes / bias folding / padding done host-side in numpy
  - recurrence matvecs: W_hh^T stationary in fp8(e3m4, x128 scale) for
    faster LDWEIGHTS (the matvec bottleneck); h streamed as bf16 (/128)
    [128,1] columns; gates land in PSUM as [128, m] so the GRU gate
    elementwise math is lane-parallel
  - GX gemms + vocab projection in bf16 (fp32 matmul is 4 cyc/row vs 1)
  - decoder h history stored k-major [128, k*(T+1)+t] in scaled bf16 so the
    projection's stationary tiles are CONTIGUOUS (FWL reads weights
    contiguously; a strided bf16 stationary operand faults the PE)
  - W_pred pre-scaled x128 so the h scale cancels
"""

import os
import sys

for _p in ("/opt/trn_rl_repo",):
    if _p not in sys.path:
        sys.path.insert(0, _p)

import numpy as np
import ml_dtypes

import concourse.bass as bass
import concourse.mybir as mybir
import concourse.tile as tile
from concourse import bacc
from concourse.bass import ds

AF = mybir.ActivationFunctionType
ALU = mybir.AluOpType
F32 = mybir.dt.float32
BF16 = mybir.dt.bfloat16
FP8 = mybir.dt.float8e3

I = 768
KP = 896          # 768 + 1 bias row, zero-padded to 7*128
H = 512           # encoder hidden
G = 3 * H         # 1536
BI = 1024         # decoder hidden
G2 = 3 * BI       # 3072
V = 28996
VP = 32768        # padded vocab (8 * 4096)
NCORES = 8
VC = VP // NCORES  # 4096

REC_FP8 = True              # fp8 e3m4 recurrence weights (else bf16)
HSCALE = 128.0 if REC_FP8 else 1.0
RW_DT = FP8 if REC_FP8 else BF16
RW_NP = ml_dtypes.float8_e3m4 if REC_FP8 else ml_dtypes.bfloat16
# keep-hot filler matmuls per step: cover the elementwise tail so the PE
# never idles (idle gaps drop the HAM clock gate to half rate)
DUMMY_ENC = int(os.environ.get("BK_DUMMY_ENC", "0"))
DUMMY_DEC = int(os.environ.get("BK_DUMMY_DEC", "0"))


def _gemm_gx(nc, pool_psum, wt_sb, xt_sb, gx_sb, n_steps, gdiv):
    """gx in t-major [128, t*gdiv + m] bf16 layout from bf16 wt/xt tiles.
    wt_sb: [128, 7, n_gates], xt_sb: [128, 7, n_steps], gx_sb: [128, n_steps*gdiv].
    """
    gxv = gx_sb[:].rearrange("p (t m) -> p t m", m=gdiv)
    n_blocks = (n_steps + 511) // 512
    for m in range(gdiv):
        for nb in range(n_blocks):
            t0 = nb * 512
            tn = min(512, n_steps - t0)
            ps = pool_psum.tile([128, 512], F32, tag="gxps")
            for k in range(7):
                nc.tensor.matmul(
                    ps[:, :tn],
                    wt_sb[:, k, m * 128:(m + 1) * 128],
                    xt_sb[:, k, t0:t0 + tn],
                    start=(k == 0),
                    stop=(k == 6),
                )
            # strided store into t-major bf16 GX
            nc.scalar.activation(gxv[:, t0:t0 + tn, m:m + 1], ps[:, :tn], AF.Copy)


def build_program(bhn_e_np, bhn_d_np, lsteps=512, losteps=1024,
                  enc_unroll=4, dec_unroll=4):
    nc = bacc.Bacc("TRN2", target_bir_lowering=False, debug=False,
                   num_devices=NCORES)

    xte = nc.dram_tensor("xte", [KP, lsteps], BF16, kind="ExternalInput")
    wte = nc.dram_tensor("wte", [KP, G], BF16, kind="ExternalInput")
    wtb = nc.dram_tensor("wtb", [KP, G], BF16, kind="ExternalInput")
    xtd = nc.dram_tensor("xtd", [KP, losteps], BF16, kind="ExternalInput")
    wtd = nc.dram_tensor("wtd", [KP, G2], BF16, kind="ExternalInput")
    whe = nc.dram_tensor("whe", [H, G], RW_DT, kind="ExternalInput")
    whd = nc.dram_tensor("whd", [BI, G2], RW_DT, kind="ExternalInput")
    wp = nc.dram_tensor("wp", [BI, VC], BF16, kind="ExternalInput")
    out = nc.dram_tensor("out", [losteps, VC], F32, kind="ExternalOutput")

    use_bhn_e = bhn_e_np is not None and np.any(bhn_e_np)
    use_bhn_d = bhn_d_np is not None and np.any(bhn_d_np)
    bhe_d = nc.inline_tensor(
        np.ascontiguousarray(bhn_e_np.reshape(4, 128).T), name="bhe") if use_bhn_e else None
    bhd_d = nc.inline_tensor(
        np.ascontiguousarray(bhn_d_np.reshape(8, 128).T), name="bhd") if use_bhn_d else None

    ENC_T = lsteps
    DEC_T = losteps
    NS = DEC_T // 128  # projection step-tiles
    INV_S = 1.0 / HSCALE
    TP = DEC_T + 1     # h history slots per k-block

    with tile.TileContext(nc) as tc:
        with tc.tile_pool(name="persist", bufs=1) as pp, \
             tc.tile_pool(name="psum_dum", bufs=1, space="PSUM") as pdum:
            # persistent state
            # hp[p, k*TP + t] = h_{t-1}[k*128 + p] / HSCALE  (bf16); slot 0 = h0
            hp = pp.tile([128, 8 * TP], BF16, tag="hp")
            ps_dum = pdum.tile([128, 4], F32, tag="ps_dum")
            h32 = [pp.tile([128, 8], F32, tag=f"h32_{j}", name=f"h32_{j}")
                   for j in range(2)]
            he_f = [pp.tile([128, 4], F32, tag=f"he_f{j}", name=f"he_f{j}")
                    for j in range(2)]
            he_b = [pp.tile([128, 4], BF16, tag=f"he_b{j}", name=f"he_b{j}")
                    for j in range(2)]
            hd_b = [pp.tile([128, 8], BF16, tag=f"hd_b{j}", name=f"hd_b{j}")
                    for j in range(2)]
            whd_sb = pp.tile([128, 8, G2], RW_DT, tag="whd_sb")
            gxd = pp.tile([128, DEC_T * 24], BF16, tag="gxd")
            bhe_sb = pp.tile([128, 4], F32, tag="bhe_sb") if use_bhn_e else None
            bhd_sb = pp.tile([128, 8], F32, tag="bhd_sb") if use_bhn_d else None
            nc.sync.dma_start(
                whd_sb[:], whd.ap().rearrange("(ko ki) g -> ki ko g", ki=128))
            if use_bhn_e:
                nc.sync.dma_start(bhe_sb[:], bhe_d[:, :])
            if use_bhn_d:
                nc.sync.dma_start(bhd_sb[:], bhd_d[:, :])

            # ---------------- Phase A: encoder GX + bw single cell ----------
            with tc.tile_pool(name="encgx", bufs=1) as pa, \
                 tc.tile_pool(name="psum_enc", bufs=2, space="PSUM") as prec:
                gxe = pa.tile([128, ENC_T * 12], BF16, tag="gxe")
                whe_sb = pa.tile([128, 4, G], RW_DT, tag="whe_sb")
                xte_sb = pa.tile([128, 7, ENC_T], BF16, tag="xte_sb")
                with tc.tile_pool(name="encw", bufs=1) as pw, \
                     tc.tile_pool(name="psum_ga", bufs=1, space="PSUM") as pgx:
                    wte_sb = pw.tile([128, 7, G], BF16, tag="wte_sb")
                    wtb_sb = pw.tile([128, 7, G], BF16, tag="wtb_sb")
                    nc.sync.dma_start(
                        xte_sb[:], xte.ap().rearrange("(ko ki) t -> ki ko t", ki=128))
                    nc.sync.dma_start(
                        wte_sb[:], wte.ap().rearrange("(ko ki) g -> ki ko g", ki=128))
                    nc.sync.dma_start(
                        wtb_sb[:], wtb.ap().rearrange("(ko ki) g -> ki ko g", ki=128))
                    nc.sync.dma_start(
                        whe_sb[:], whe.ap().rearrange("(ko ki) g -> ki ko g", ki=128))

                    _gemm_gx(nc, pgx, wte_sb, xte_sb, gxe, ENC_T, 12)

                    # backward encoder: single cell on x[last], h0 = 0
                    # gx_b = wtb_aug.T @ x_aug[:, last]  -> [128, 12] psum
                    ps_b = pgx.tile([128, 12], F32, tag="ps_b")
                    for m in range(12):
                        for k in range(7):
                            nc.tensor.matmul(
                                ps_b[:, m:m + 1],
                                wtb_sb[:, k, m * 128:(m + 1) * 128],
                                xte_sb[:, k, ENC_T - 1:ENC_T],
                                start=(k == 0),
                                stop=(k == 6),
                            )
                    # z = sigmoid(gx_z); n = tanh(gx_n [+ r*bhn]); h = (1-z)*n
                    zb = pa.tile([128, 4], F32, tag="zb")
                    nb = pa.tile([128, 4], F32, tag="nb")
                    tb = pa.tile([128, 4], F32, tag="tb")
                    nc.scalar.activation(zb[:], ps_b[:, 4:8], AF.Sigmoid)
                    if use_bhn_e:
                        rb = pa.tile([128, 4], F32, tag="rb")
                        nc.scalar.activation(rb[:], ps_b[:, 0:4], AF.Sigmoid)
                        nc.vector.tensor_mul(tb[:], rb[:], bhe_sb[:])
                        nc.vector.tensor_add(tb[:], tb[:], ps_b[:, 8:12])
                        nc.scalar.activation(nb[:], tb[:], AF.Tanh)
                    else:
                        nc.scalar.activation(nb[:], ps_b[:, 8:12], AF.Tanh)
                    nc.vector.tensor_scalar(
                        out=zb[:], in0=zb[:], scalar1=-1.0, scalar2=1.0,
                        op0=ALU.mult, op1=ALU.add)
                    # h_bw -> h32[0] cols 4:8
                    nc.vector.tensor_mul(h32[0][:, 4:8], zb[:], nb[:])

                # ---------------- Phase B: forward encoder recurrence -------
                nc.vector.memset(he_f[0][:], 0.0)
                nc.vector.memset(he_b[0][:], 0.0)
                gxev = gxe[:]
                with tc.For_i(0, ENC_T, enc_unroll,
                              hint_engines=(mybir.EngineType.PE,)) as iv:
                    for u in range(enc_unroll):
                        t = iv + u
                        cur = u % 2
                        nxt = (u + 1) % 2
                        ps_rz = prec.tile([128, 8], F32, tag="ps_rz_e")
                        ps_n = prec.tile([128, 4], F32, tag="ps_n_e")
                        for m in range(12):
                            tgt = ps_rz[:, m:m + 1] if m < 8 else ps_n[:, m - 8:m - 7]
                            for k in range(4):
                                nc.tensor.matmul(
                                    tgt,
                                    whe_sb[:, k, m * 128:(m + 1) * 128],
                                    he_b[cur][:, k:k + 1],
                                    start=(k == 0), stop=(k == 3))
                        grz = pa.tile([128, 8], F32, tag=f"grz_e{u}")
                        rz = pa.tile([128, 8], F32, tag=f"rz_e{u}")
                        aa = pa.tile([128, 4], F32, tag=f"aa_e{u}")
                        omz = pa.tile([128, 4], F32, tag=f"omz_e{u}")
                        t1 = pa.tile([128, 4], F32, tag=f"t1_e{u}")
                        bn = pa.tile([128, 4], F32, tag=f"bn_e{u}")
                        nn = pa.tile([128, 4], F32, tag=f"nn_e{u}")
                        for j in range(DUMMY_ENC):
                            nc.tensor.matmul(
                                ps_dum[:, j % 4:j % 4 + 1],
                                whe_sb[:, 0, 0:128], gxev[:, 0:1],
                                start=True, stop=True)
                        nc.vector.tensor_add(grz[:], ps_rz[:], gxev[:, ds(t * 12, 8)])
                        nc.scalar.activation(rz[:], grz[:], AF.Sigmoid)
                        # early (overlaps n-gate matvecs): a = z*h, 1-z
                        nc.vector.tensor_mul(aa[:], rz[:, 4:8], he_f[cur][:])
                        nc.vector.tensor_scalar(
                            out=omz[:], in0=rz[:, 4:8], scalar1=-1.0, scalar2=1.0,
                            op0=ALU.mult, op1=ALU.add)
                        if use_bhn_e:
                            nc.vector.tensor_add(ps_n[:], ps_n[:], bhe_sb[:])
                        nc.vector.tensor_mul(t1[:], rz[:, 0:4], ps_n[:])
                        nc.vector.tensor_add(t1[:], t1[:], gxev[:, ds(t * 12 + 8, 4)])
                        nc.scalar.activation(nn[:], t1[:], AF.Tanh)
                        # h_new = (1-z)*n + z*h ; scaled bf16 copy unblocks PE
                        nc.vector.tensor_mul(bn[:], omz[:], nn[:])
                        nc.vector.tensor_add(he_f[nxt][:], bn[:], aa[:])
                        nc.vector.tensor_scalar_mul(he_b[nxt][:], he_f[nxt][:], INV_S)
                # final fw state is in he_f[0] (ENC_T multiple of 2)
                nc.vector.tensor_copy(h32[0][:, 0:4], he_f[0][:])

            # h0 for decoder: scaled bf16 into hp slot 0 (stride TP) + ping-pong
            nc.vector.tensor_scalar_mul(hp[:, ds(0, 8, TP)], h32[0][:], INV_S)
            nc.vector.tensor_scalar_mul(hd_b[0][:], h32[0][:], INV_S)

            # ---------------- Phase C: decoder GX ---------------------------
            with tc.tile_pool(name="decgx", bufs=1) as pc, \
                 tc.tile_pool(name="wtd_stream", bufs=3) as pwtd, \
                 tc.tile_pool(name="psum_gc", bufs=2, space="PSUM") as pgx:
                xtd_sb = pc.tile([128, 7, DEC_T], BF16, tag="xtd_sb")
                nc.sync.dma_start(
                    xtd_sb[:], xtd.ap().rearrange("(ko ki) t -> ki ko t", ki=128))
                gxdv = gxd[:].rearrange("p (t m) -> p t m", m=24)
                n_blocks = (DEC_T + 511) // 512
                for m in range(24):
                    wtd_t = pwtd.tile([128, 7, 128], BF16, tag="wtd_t")
                    nc.sync.dma_start(
                        wtd_t[:],
                        wtd.ap()[:, m * 128:(m + 1) * 128].rearrange(
                            "(ko ki) g -> ki ko g", ki=128))
                    for nb in range(n_blocks):
                        t0 = nb * 512
                        tn = min(512, DEC_T - t0)
                        ps = pgx.tile([128, 512], F32, tag="gxps")
                        for k in range(7):
                            nc.tensor.matmul(
                                ps[:, :tn], wtd_t[:, k, :], xtd_sb[:, k, t0:t0 + tn],
                                start=(k == 0), stop=(k == 6))
                        nc.scalar.activation(gxdv[:, t0:t0 + tn, m:m + 1],
                                             ps[:, :tn], AF.Copy)

            # ---------------- Phase D: decoder recurrence -------------------
            with tc.tile_pool(name="dec_sc", bufs=1) as pd, \
                 tc.tile_pool(name="psum_dec", bufs=2, space="PSUM") as prec:
                with tc.For_i(0, DEC_T, dec_unroll,
                              hint_engines=(mybir.EngineType.PE,)) as iv:
                    for u in range(dec_unroll):
                        t = iv + u
                        cur = u % 2
                        nxt = (u + 1) % 2
                        ps_rz = prec.tile([128, 16], F32, tag="ps_rz_d")
                        ps_n = prec.tile([128, 8], F32, tag="ps_n_d")
                        for m in range(24):
                            tgt = (ps_rz[:, m:m + 1] if m < 16
                                   else ps_n[:, m - 16:m - 15])
                            for k in range(8):
                                nc.tensor.matmul(
                                    tgt,
                                    whd_sb[:, k, m * 128:(m + 1) * 128],
                                    hd_b[cur][:, k:k + 1],
                                    start=(k == 0), stop=(k == 7))
                        grz = pd.tile([128, 16], F32, tag=f"grz_d{u}")
                        rz = pd.tile([128, 16], F32, tag=f"rz_d{u}")
                        aa = pd.tile([128, 8], F32, tag=f"aa_d{u}")
                        a1 = pd.tile([128, 8], F32, tag=f"a1_d{u}")
                        omz = pd.tile([128, 8], F32, tag=f"omz_d{u}")
                        t1 = pd.tile([128, 8], F32, tag=f"t1_d{u}")
                        bn = pd.tile([128, 8], F32, tag=f"bn_d{u}")
                        nn = pd.tile([128, 8], F32, tag=f"nn_d{u}")
                        for j in range(DUMMY_DEC):
                            nc.tensor.matmul(
                                ps_dum[:, j % 4:j % 4 + 1],
                                whd_sb[:, 0, 0:128], gxd[:][:, 0:1],
                                start=True, stop=True)
                        nc.vector.tensor_add(grz[:], ps_rz[:],
                                             gxd[:][:, ds(t * 24, 16)])
                        nc.scalar.activation(rz[:], grz[:], AF.Sigmoid)
                        # early (overlap n matvecs): a = z*h, a1 = a/S, 1-z
                        nc.vector.tensor_mul(aa[:], rz[:, 8:16], h32[cur][:])
                        nc.vector.tensor_scalar_mul(a1[:], aa[:], INV_S)
                        nc.vector.tensor_scalar(
                            out=omz[:], in0=rz[:, 8:16], scalar1=-1.0,
                            scalar2=1.0, op0=ALU.mult, op1=ALU.add)
                        if use_bhn_d:
                            nc.vector.tensor_add(ps_n[:], ps_n[:], bhd_sb[:])
                        nc.vector.tensor_mul(t1[:], rz[:, 0:8], ps_n[:])
                        nc.vector.tensor_add(t1[:], t1[:],
                                             gxd[:][:, ds(t * 24 + 16, 8)])
                        nc.scalar.activation(nn[:], t1[:], AF.Tanh)
                        # critical: hd_b[nxt] = bn/S + a1 unblocks next step
                        nc.vector.tensor_mul(bn[:], omz[:], nn[:])
                        nc.vector.scalar_tensor_tensor(
                            hd_b[nxt][:], bn[:], INV_S, a1[:],
                            ALU.mult, ALU.add)
                        # off-critical: projection history + f32 h carry
                        nc.vector.tensor_copy(hp[:, ds(t + 1, 8, TP)],
                                              hd_b[nxt][:])
                        nc.vector.tensor_add(h32[nxt][:], bn[:], aa[:])

            # ---------------- Phase E: vocab projection ---------------------
            with tc.tile_pool(name="wp_pool", bufs=2) as pwp, \
                 tc.tile_pool(name="out_pool", bufs=3) as pout, \
                 tc.tile_pool(name="psum_o", bufs=4, space="PSUM") as pso:
                for n in range(VC // 512):
                    wpn = pwp.tile([128, 8, 512], BF16, tag="wpn")
                    nc.sync.dma_start(
                        wpn[:],
                        wp.ap()[:, n * 512:(n + 1) * 512].rearrange(
                            "(ko ki) v -> ki ko v", ki=128))
                    for s in range(NS):
                        ps = pso.tile([128, 512], F32, tag="ps_o")
                        for k in range(8):
                            # contiguous bf16 stationary tile (FWL-safe)
                            nc.tensor.matmul(
                                ps[:],
                                hp[:, k * TP + 1 + s * 128:
                                   k * TP + 1 + (s + 1) * 128],
                                wpn[:, k, :],
                                start=(k == 0), stop=(k == 7))
                        ot = pout.tile([128, 512], F32, tag="ot")
                        nc.vector.tensor_copy(ot[:], ps[:])
                        nc.sync.dma_start(
                            out.ap()[s * 128:(s + 1) * 128,
                                     n * 512:(n + 1) * 512], ot[:])

    nc.compile()
    return nc


def _prep_inputs(inputs, lsteps=512, losteps=1024):
    f = lambda k: np.asarray(inputs[k], np.float32)
    bf = lambda a: np.ascontiguousarray(a).astype(ml_dtypes.bfloat16)
    x = f("input_context")[:lsteps]
    oc = f("output_context")[:losteps]
    dec_in = np.concatenate([oc[:1], oc[:-1]], axis=0)

    def aug_x(xT_cols):
        a = np.zeros((KP, xT_cols.shape[1]), np.float32)
        a[:I] = xT_cols
        a[I] = 1.0
        return a

    def aug_w(wih, bih, bhh, hh):
        a = np.zeros((KP, 3 * hh), np.float32)
        a[:I] = wih.T
        bias = bih.copy()
        bias[:2 * hh] += bhh[:2 * hh]
        a[I] = bias
        return a

    def rq(whh):  # recurrence weight: transpose, scale, quantize
        w = np.ascontiguousarray(whh.T) * HSCALE
        if REC_FP8:
            w = np.clip(w, -15.5, 15.5)
        return w.astype(RW_NP)

    xte = bf(aug_x(x.T))
    xtd = bf(aug_x(dec_in.T))
    wte = bf(aug_w(f("fw_wih"), f("fw_bih"), f("fw_bhh"), H))
    wtb = bf(aug_w(f("bw_wih"), f("bw_bih"), f("bw_bhh"), H))
    wtd = bf(aug_w(f("dec_wih"), f("dec_bih"), f("dec_bhh"), BI))
    whe = rq(f("fw_whh"))
    whd = rq(f("dec_whh"))
    wp_pad = np.zeros((BI, VP), np.float32)
    wp_pad[:, :V] = f("W_pred") * HSCALE
    wp_pad = wp_pad.astype(ml_dtypes.bfloat16)
    bhn_e = f("fw_bhh")[2 * H:]
    bhn_d = f("dec_bhh")[2 * BI:]

    common = dict(xte=xte, wte=wte, wtb=wtb, xtd=xtd, wtd=wtd, whe=whe,
                  whd=whd)
    in_maps = [dict(common, wp=np.ascontiguousarray(wp_pad[:, c * VC:(c + 1) * VC]))
               for c in range(NCORES)]
    return in_maps, bhn_e, bhn_d


def _bench_pjrt(nc, in_maps, warmup=2, iters=10):
    """Time repeated on-device executions with device-resident inputs.

    Returns (per_call_min_s, burst_avg_s): per-call = block every call
    (exec + RTT); burst = async dispatch of `iters` calls, block once
    (amortizes dispatch; ~max(exec, dispatch)).
    """
    import time

    import jax
    from jax.sharding import Mesh, PartitionSpec
    from jax.experimental.shard_map import shard_map

    from concourse import bass2jax
    from concourse.bass2jax import _bass_exec_p, install_neuronx_cc_hook

    install_neuronx_cc_hook()
    partition_name = (nc.partition_id_tensor.name
                      if nc.partition_id_tensor else None)
    in_names, out_names, out_avals, zero_outs = [], [], [], []
    for alloc in nc.m.functions[0].allocations:
        if not isinstance(alloc, mybir.MemoryLocationSet):
            continue
        name = alloc.memorylocations[0].name
        if alloc.kind == "ExternalInput":
            if name != partition_name:
                in_names.append(name)
        elif alloc.kind == "ExternalOutput":
            shape = tuple(alloc.tensor_shape)
            dtype = mybir.dt.np(alloc.dtype)
            out_names.append(name)
            out_avals.append(jax.core.ShapedArray(shape, dtype))
            zero_outs.append(np.zeros(shape, dtype))
    n_params = len(in_names)
    all_names = list(in_names) + list(out_names)
    if partition_name is not None:
        all_names.append(partition_name)

    def _body(*args):
        operands = list(args)
        if partition_name is not None:
            operands.append(bass2jax.partition_id_tensor())
        return tuple(_bass_exec_p.bind(
            *operands,
            out_avals=tuple(out_avals),
            in_names=tuple(all_names),
            out_names=tuple(out_names),
            lowering_input_output_aliases=(),
            sim_require_finite=False,
            sim_require_nnan=False,
            nc=nc,
        ))

    devices = jax.devices()[:NCORES]
    mesh = Mesh(np.asarray(devices), ("core",))
    nin = n_params + len(zero_outs)
    fn = jax.jit(shard_map(
        _body, mesh=mesh,
        in_specs=(PartitionSpec("core"),) * nin,
        out_specs=(PartitionSpec("core"),) * len(out_names),
        check_rep=False))
    sh = jax.sharding.NamedSharding(mesh, PartitionSpec("core"))
    dev_in = [
        jax.device_put(
            np.concatenate([np.asarray(in_maps[c][nm]) for c in range(NCORES)],
                           axis=0), sh)
        for nm in in_names
    ] + [
        jax.device_put(
            np.zeros((NCORES * z.shape[0], *z.shape[1:]), z.dtype), sh)
        for z in zero_outs
    ]
    for _ in range(warmup):
        jax.block_until_ready(fn(*dev_in))
    times = []
    for _ in range(iters):
        t0 = time.perf_counter()
        jax.block_until_ready(fn(*dev_in))
        times.append(time.perf_counter() - t0)
    t0 = time.perf_counter()
    outs = None
    for _ in range(iters):
        outs = fn(*dev_in)
    jax.block_until_ready(outs)
    burst = (time.perf_counter() - t0) / iters
    return min(times), burst


_CACHE = {}
LAST_EXEC_NS = None


def kernel(**inputs) -> np.ndarray:
    global LAST_EXEC_NS

    from concourse import bass_utils

    in_maps, bhn_e, bhn_d = _prep_inputs(inputs)
    key = (bool(np.any(bhn_e)), bool(np.any(bhn_d)))
    if key not in _CACHE:
        _CACHE[key] = build_program(bhn_e, bhn_d)
    nc = _CACHE[key]
    res = bass_utils.run_bass_kernel_spmd(
        nc, in_maps, core_ids=list(range(NCORES)))
    LAST_EXEC_NS = res.exec_time_ns
    if os.environ.get("BASS_KERNEL_BENCH", "0") == "1":
        per_call, burst = _bench_pjrt(nc, in_maps)
        LAST_EXEC_NS = int(burst * 1e9)
        print(f"bench: per-call min {per_call*1e3:.3f} ms, "
              f"burst avg {burst*1e3:.3f} ms")
    preds = np.concatenate([res.results[c]["out"] for c in range(NCORES)],
                           axis=1)
    return np.ascontiguousarray(preds[:, :V]).astype(np.float32)


if __name__ == "__main__":
    # smoke test with random inputs
    rng = np.random.default_rng(0)
    inp = {
        "input_context": rng.standard_normal((512, I), dtype=np.float32),
        "output_context": rng.standard_normal((1024, I), dtype=np.float32),
        "fw_wih": rng.standard_normal((G, I), dtype=np.float32) * 0.02,
        "fw_whh": rng.standard_normal((G, H), dtype=np.float32) * 0.02,
        "fw_bih": np.zeros(G, np.float32), "fw_bhh": np.zeros(G, np.float32),
        "bw_wih": rng.standard_normal((G, I), dtype=np.float32) * 0.02,
        "bw_whh": rng.standard_normal((G, H), dtype=np.float32) * 0.02,
        "bw_bih": np.zeros(G, np.float32), "bw_bhh": np.zeros(G, np.float32),
        "dec_wih": rng.standard_normal((G2, I), dtype=np.float32) * 0.02,
        "dec_whh": rng.standard_normal((G2, BI), dtype=np.float32) * 0.02,
        "dec_bih": np.zeros(G2, np.float32), "dec_bhh": np.zeros(G2, np.float32),
        "W_pred": rng.standard_normal((BI, V), dtype=np.float32) * 0.02,
    }
    out = kernel(**inp)
    print("out", out.shape, out.dtype, float(np.abs(out).max()))
